# revision 15
# baseline (speedup 1.0000x reference)
"""Bayesian masked 2-layer MLP (MC mean/std) on 8 Trainium2 NeuronCores.

Strategy (tensor-parallel over out-features, per sharding hint):
- The MADE mask is block-upper-triangular at 64 granularity; at 128x128
  "superblock" granularity, out-superblock s only contracts input features
  >= 128*s (plus one zeroed 64x64 corner on the diagonal tile). We skip the
  masked region structurally: weights/eps are shipped pre-masked and only
  the live K-range is read/computed.
- 32 out-feature superblocks; core c owns s in {c, 8+c, 16+c, 24+c}
  (chunk j -> s = 8j + c). To keep one SPMD program for all cores, chunk j
  contracts K-tiles t in [8j, 32) on every core (small zero-padding for
  cores whose s > 8j).
- Layer 1 uses the two-matmul split  x @ W1_k^T = x @ mu1^T + x @ (sig1*eps1_k)^T
  so the mu1 matmul runs once (U), not per MC sample, and no W1 assembly
  add is needed. Layer 2 assembles W2_k = mu2 + sig2*eps2_k on the vector
  engine (mu2/sig resident in SBUF).
- h_k is produced feature-major ([feat, batch]) directly by making weights
  the stationary matmul operand, so layer 2 consumes it with no transpose.
  Shards are AllGathered across the 8 cores per MC sample.
- mean/std (ddof=1) over the 5 samples computed on-device (two-pass).

Epsilons: the reference draws them with jax.random inside the model; they
are reproduced here with the identical jax calls (functionally pure ->
identical values on this jax stack) and streamed in as inputs.
"""
import os
import sys

for _p in ("/opt/trn_rl_repo", "/root/.axon_site/_ro/trn_rl_repo"):
    if _p not in sys.path and os.path.isdir(_p):
        sys.path.append(_p)

import numpy as np

B = 64          # batch
D = 4096        # flattened feature dim
NMC = 5         # MC samples
NCORES = 8
NSB = 32        # superblocks of 128 out-features
JCH = 4         # superblock chunks per core
T0 = [0, 8, 16, 24]          # first K-tile per chunk (uniform across cores)
NT = [32, 24, 16, 8]         # K-tiles per chunk
TILE_OFF = [0, 32, 56, 72]   # tile offset of chunk j in the packed [80,...]
NTILES = 80
GRP = 32                     # K-tiles per DMA/DVE group
BF16 = True                  # bf16 matmul operands (PSUM accum stays fp32)

_cache = {}


# ---------------------------------------------------------------- device code

def _build_bass():
    from concourse import bacc, tile
    import concourse.mybir as mybir

    f32 = mybir.dt.float32
    bf16 = mybir.dt.bfloat16
    wdt = bf16 if BF16 else f32
    nc = bacc.Bacc("TRN2", target_bir_lowering=False, debug=False,
                   num_devices=NCORES)

    def din(name, shape, dt=f32):
        return nc.dram_tensor(name, shape, dt, kind="ExternalInput").ap()

    xT = din("xT", [128, 32 * B], wdt)
    w1mu = din("w1mu", [128, NTILES * 128], wdt)
    w2mu = din("w2mu", [128, NTILES * 128], wdt)
    e1 = din("e1", [NMC, 128, NTILES * 128], wdt)
    e2 = din("e2", [NMC, 128, NTILES * 128], wdt)
    bmu1 = din("bmu1", [128, JCH])
    brho1 = din("brho1", [128, JCH])
    bmu2 = din("bmu2", [128, JCH])
    brho2 = din("brho2", [128, JCH])
    eb1 = din("eb1", [NMC, 128, JCH])
    eb2 = din("eb2", [NMC, 128, JCH])
    mean_o = nc.dram_tensor("mean", [JCH * 128, B], f32, kind="ExternalOutput").ap()
    std_o = nc.dram_tensor("std", [JCH * 128, B], f32, kind="ExternalOutput").ap()

    EXP = mybir.ActivationFunctionType.Exp
    LN = mybir.ActivationFunctionType.Ln
    RELU = mybir.ActivationFunctionType.Relu
    SQRT = mybir.ActivationFunctionType.Sqrt
    COPY = mybir.ActivationFunctionType.Copy
    MULT = mybir.AluOpType.mult

    # groups of K-tiles: (chunk j, packed tile offset, group size, t of first)
    groups = []
    for j in range(JCH):
        t = T0[j]
        while t < 32:
            g = min(GRP, 32 - t)
            groups.append((j, TILE_OFF[j] + (t - T0[j]), g, t))
            t += g

    with tile.TileContext(nc) as tc:
        with (
            tc.tile_pool(name="const", bufs=1) as cpool,
            tc.tile_pool(name="stream", bufs=3) as spool,
            tc.tile_pool(name="e1stream", bufs=5) as e1pool,
            tc.tile_pool(name="e2stream", bufs=3) as e2pool,
            tc.tile_pool(name="assemble", bufs=3) as apool,
            tc.tile_pool(name="small", bufs=3) as mpool,
            tc.tile_pool(name="hts", bufs=3) as hpool,
            tc.tile_pool(name="htf", bufs=3) as fpool,
            tc.tile_pool(name="psum", bufs=8, space="PSUM") as ppool,
            tc.tile_pool(name="dram", bufs=NMC, space="DRAM") as dpool,
        ):
            # ---- resident tiles
            xts = cpool.tile([128, 32 * B], wdt)
            mu2s = cpool.tile([128, NTILES * 128], wdt)
            U = cpool.tile([128, JCH * B], f32)            # 1KB/part
            y = cpool.tile([128, NMC * JCH * B], f32)      # 5KB/part
            bc = cpool.tile([128, 8 * JCH], f32)  # bmu1|bmu2|sigb1|sigb2 ...
            ebc1 = cpool.tile([128, NMC * JCH], f32)
            ebc2 = cpool.tile([128, NMC * JCH], f32)

            nc.sync.dma_start(xts[:], xT[:, :])
            nc.sync.dma_start(ebc1[:].rearrange("p (k j) -> p k j", k=NMC),
                              eb1.rearrange("k p j -> p k j"))
            nc.sync.dma_start(ebc2[:].rearrange("p (k j) -> p k j", k=NMC),
                              eb2.rearrange("k p j -> p k j"))
            nc.sync.dma_start(bc[:, 0:JCH], bmu1[:, :])
            nc.sync.dma_start(bc[:, JCH:2 * JCH], bmu2[:, :])
            # softplus(brho) -> bc[:, 2J:3J], bc[:, 3J:4J]
            tmpb = mpool.tile([128, 2 * JCH], f32, tag="tmpb")
            nc.sync.dma_start(tmpb[:, 0:JCH], brho1[:, :])
            nc.sync.dma_start(tmpb[:, JCH:2 * JCH], brho2[:, :])
            # softplus(x) = Ln(Exp(x) + 1)  (Softplus has no ACT table on gen3)
            tmpe = mpool.tile([128, 2 * JCH], f32, tag="tmpe")
            nc.scalar.activation(tmpe[:], tmpb[:], EXP)
            nc.scalar.activation(bc[:, 2 * JCH:3 * JCH], tmpe[:, 0:JCH], LN,
                                 bias=1.0)
            nc.scalar.activation(bc[:, 3 * JCH:4 * JCH], tmpe[:, JCH:2 * JCH],
                                 LN, bias=1.0)

            # ---- sigma = softplus(rho) resident; mu2 resident
            nc.gpsimd.dma_start(mu2s[:], w2mu[:, :])

            # ---- U = x @ mu1^T  (feature-major psum [outfeat, batch])
            pU = ppool.tile([128, JCH * B], f32, tag="ps")
            for (j, off, g, t) in groups:
                mt = spool.tile([128, GRP * 128], wdt, tag="ld")
                nc.gpsimd.dma_start(
                    mt[:, : g * 128],
                    w1mu[:, off * 128:(off + g) * 128],
                )
                for i in range(g):
                    tt = t + i
                    nc.tensor.matmul(
                        pU[:, j * B:(j + 1) * B],
                        mt[:, i * 128:(i + 1) * 128],
                        xts[:, tt * B:(tt + 1) * B],
                        start=(tt == T0[j]),
                        stop=(tt == 31),
                    )
            nc.scalar.activation(U[:], pU[:], COPY)

            # ---- per-sample phases
            def bias_k(k, which):
                # b_k = bmu + sigb * eb_k   -> [128, JCH] tile
                bt = mpool.tile([128, JCH], f32, tag="bk")
                ebc = ebc1 if which == 1 else ebc2
                mu_off = 0 if which == 1 else JCH
                sg_off = 2 * JCH if which == 1 else 3 * JCH
                nc.vector.tensor_tensor(
                    bt[:], ebc[:, k * JCH:(k + 1) * JCH],
                    bc[:, sg_off:sg_off + JCH], MULT)
                nc.vector.tensor_add(bt[:], bt[:], bc[:, mu_off:mu_off + JCH])
                return bt

            def layer1(k):
                b1k = bias_k(k, 1)
                hts = hpool.tile([128, JCH * B], wdt, tag="hts")
                pV = ppool.tile([128, JCH * B], f32, tag="ps")
                for (j, off, g, t) in groups:
                    et = e1pool.tile([128, GRP * 128], wdt, tag="e1")
                    nc.sync.dma_start(
                        et[:, : g * 128],
                        e1[k][:, off * 128:(off + g) * 128],
                    )
                    for i in range(g):
                        tt = t + i
                        nc.tensor.matmul(
                            pV[:, j * B:(j + 1) * B],
                            et[:, i * 128:(i + 1) * 128],
                            xts[:, tt * B:(tt + 1) * B],
                            start=(tt == T0[j]),
                            stop=(tt == 31),
                        )
                pre = mpool.tile([128, JCH * B], f32, tag="pre")
                nc.vector.tensor_add(pre[:], pV[:], U[:])
                for j in range(JCH):
                    nc.scalar.activation(hts[:, j * B:(j + 1) * B],
                                         pre[:, j * B:(j + 1) * B],
                                         RELU, bias=b1k[:, j:j + 1])
                return hts

            def gather(k, hts):
                gin = dpool.tile([JCH * 128, B], wdt, tag="gin")
                gout = dpool.tile([NSB * 128, B], wdt, tag="gout")
                nc.sync.dma_start(
                    gin.rearrange("(j p) n -> p j n", p=128),
                    hts[:].rearrange("p (j n) -> p j n", j=JCH),
                )
                nc.gpsimd.collective_compute(
                    "AllGather",
                    mybir.AluOpType.bypass,
                    ins=[gin.opt()],
                    outs=[gout.opt()],
                    replica_groups=[list(range(NCORES))],
                )
                htf = fpool.tile([128, NSB * B], wdt, tag="htf")
                # gout rows = 512*r + 128*j + p  <->  global K-tile t = 8j + r
                # htf free layout: chunk t at cols [t*B, (t+1)*B) = (j r n);
                # DMA APs are limited to 3 dims, so one DMA per source rank.
                for r in range(NCORES):
                    nc.sync.dma_start(
                        htf[:].rearrange("p (j rn) -> p j rn", j=JCH)[
                            :, :, r * B:(r + 1) * B],
                        gout[512 * r:512 * (r + 1)].rearrange(
                            "(j p) n -> p j n", p=128),
                    )
                return htf

            def layer2(k, htf):
                b2k = bias_k(k, 2)
                pY = ppool.tile([128, JCH * B], f32, tag="ps")
                for (j, off, g, t) in groups:
                    et = e2pool.tile([128, GRP * 128], wdt, tag="e2")
                    nc.scalar.dma_start(
                        et[:, : g * 128],
                        e2[k][:, off * 128:(off + g) * 128],
                    )
                    wt = apool.tile([128, GRP * 128], wdt, tag="RW")
                    nc.vector.tensor_add(
                        wt[:, : g * 128], et[:, : g * 128],
                        mu2s[:, off * 128:(off + g) * 128])
                    for i in range(g):
                        tt = t + i
                        nc.tensor.matmul(
                            pY[:, j * B:(j + 1) * B],
                            wt[:, i * 128:(i + 1) * 128],
                            htf[:, tt * B:(tt + 1) * B],
                            start=(tt == T0[j]),
                            stop=(tt == 31),
                        )
                for j in range(JCH):
                    nc.vector.tensor_scalar_add(
                        y[:, (k * JCH + j) * B:(k * JCH + j + 1) * B],
                        pY[:, j * B:(j + 1) * B], b2k[:, j:j + 1])

            # software pipeline, depth 2: L1 runs two samples ahead so the
            # AllGather latency of sample k hides under L1(k+1)/L1(k+2)
            htss = [None] * NMC
            htss[0] = layer1(0)
            htss[1] = layer1(1)
            htfs = [None] * NMC
            for k in range(NMC):
                htfs[k] = gather(k, htss[k])
                if k + 2 < NMC:
                    htss[k + 2] = layer1(k + 2)
                layer2(k, htfs[k])

            # ---- mean / std over samples
            def ysl(k, j):
                return y[:, (k * JCH + j) * B:(k * JCH + j + 1) * B]

            for j in range(JCH):
                m = mpool.tile([128, B], f32, tag="m")
                t1 = mpool.tile([128, B], f32, tag="t1")
                t2 = mpool.tile([128, B], f32, tag="t2")
                nc.vector.tensor_add(t1[:], ysl(0, j), ysl(1, j))
                nc.vector.tensor_add(t2[:], ysl(2, j), ysl(3, j))
                nc.vector.tensor_add(t1[:], t1[:], t2[:])
                nc.vector.tensor_add(t1[:], t1[:], ysl(4, j))
                nc.vector.tensor_scalar_mul(m[:], t1[:], 1.0 / NMC)
                nc.sync.dma_start(mean_o.rearrange("(j p) n -> p j n", p=128)[:, j],
                                  m[:])
                acc = mpool.tile([128, B], f32, tag="acc")
                d = mpool.tile([128, B], f32, tag="d")
                nc.vector.tensor_sub(d[:], ysl(0, j), m[:])
                nc.vector.tensor_tensor(acc[:], d[:], d[:], MULT)
                for k in range(1, NMC):
                    dk = mpool.tile([128, B], f32, tag="d")
                    sq = mpool.tile([128, B], f32, tag="sq")
                    nc.vector.tensor_sub(dk[:], ysl(k, j), m[:])
                    nc.vector.tensor_tensor(sq[:], dk[:], dk[:], MULT)
                    nc.vector.tensor_add(acc[:], acc[:], sq[:])
                nc.vector.tensor_scalar_mul(acc[:], acc[:], 1.0 / (NMC - 1))
                std_t = mpool.tile([128, B], f32, tag="stdt")
                nc.scalar.activation(std_t[:], acc[:], SQRT)
                nc.sync.dma_start(std_o.rearrange("(j p) n -> p j n", p=128)[:, j],
                                  std_t[:])

    nc.compile()
    return nc


# ---------------------------------------------------------------- host prep

def _gen_eps():
    """Reproduce the reference's jax.random draws (identical calls/keys)."""
    import jax

    keys = jax.random.split(jax.random.key(42), NMC * 4).reshape(NMC, 4)

    def one_mc_eps(k):
        return (
            jax.random.normal(k[0], (D, D)),
            jax.random.normal(k[1], (D,)),
            jax.random.normal(k[2], (D, D)),
            jax.random.normal(k[3], (D,)),
        )

    ew1, ebs1, ew2, ebs2 = jax.vmap(one_mc_eps)(keys)
    return (np.asarray(ew1), np.asarray(ebs1),
            np.asarray(ew2), np.asarray(ebs2))


def _pack_weight(w, c, masked):
    """Pack [D, D] (out, in) -> SBUF image [128, NTILES*128]: partition p =
    within-K-tile row, free = (tile, outfeat-local). Chunk j covers
    superblock s=8j+c, K-tiles t in [T0[j], 32); tiles t < s stay zero
    (mask padding); diagonal corner zeroed if masked."""
    out = np.zeros((NTILES, 128, 128), np.float32)
    blk = w.reshape(NSB, 128, NSB, 128)  # [s_out, f, t_in, p]
    for j in range(JCH):
        s = 8 * j + c
        # copy tiles t in [s, 32): blk[s, :, s:32, :] -> [t, p, f]
        src = blk[s, :, s:32, :].transpose(1, 2, 0)
        out[TILE_OFF[j] + (s - T0[j]): TILE_OFF[j] + NT[j]] = src
        if masked:
            out[TILE_OFF[j] + (s - T0[j]), 0:64, 64:128] = 0.0
    return np.ascontiguousarray(out.transpose(1, 0, 2)).reshape(128, NTILES * 128)


def _pack_bias(v, c):
    """[D] -> [128, JCH]: column j = superblock 8j+c."""
    return np.ascontiguousarray(
        v.reshape(NSB, 128)[[8 * j + c for j in range(JCH)], :].T
    ).astype(np.float32)


def _prep_in_maps(inputs):
    import ml_dtypes
    wdt = ml_dtypes.bfloat16 if BF16 else np.float32

    x = np.asarray(inputs["x"], np.float32)
    xTf = x.reshape(B, D).T.reshape(32, 128, B)
    xT = np.ascontiguousarray(xTf.transpose(1, 0, 2)).reshape(128, 32 * B).astype(wdt)

    if "eps" not in _cache:
        _cache["eps"] = _gen_eps()
    ew1, ebs1, ew2, ebs2 = _cache["eps"]

    mu1 = np.asarray(inputs["weight_mu1"], np.float32)
    rho1 = np.asarray(inputs["weight_rho1"], np.float32)
    mu2 = np.asarray(inputs["weight_mu2"], np.float32)
    rho2 = np.asarray(inputs["weight_rho2"], np.float32)
    sig1 = np.log1p(np.exp(rho1))
    sig2 = np.log1p(np.exp(rho2))

    in_maps = []
    for c in range(NCORES):
        m = {
            "xT": xT,
            "w1mu": _pack_weight(mu1, c, masked=True).astype(wdt),
            "w2mu": _pack_weight(mu2, c, masked=True).astype(wdt),
            "e1": np.stack([_pack_weight(ew1[k] * sig1, c, masked=True)
                            .astype(wdt) for k in range(NMC)]),
            "e2": np.stack([_pack_weight(ew2[k] * sig2, c, masked=True)
                            .astype(wdt) for k in range(NMC)]),
            "bmu1": _pack_bias(np.asarray(inputs["bias_mu1"], np.float32), c),
            "brho1": _pack_bias(np.asarray(inputs["bias_rho1"], np.float32), c),
            "bmu2": _pack_bias(np.asarray(inputs["bias_mu2"], np.float32), c),
            "brho2": _pack_bias(np.asarray(inputs["bias_rho2"], np.float32), c),
            "eb1": np.stack([_pack_bias(ebs1[k], c) for k in range(NMC)]),
            "eb2": np.stack([_pack_bias(ebs2[k], c) for k in range(NMC)]),
        }
        in_maps.append(m)
    return in_maps


def kernel(**inputs):
    import concourse.bass_utils as bass_utils

    if "nc" not in _cache:
        _cache["nc"] = _build_bass()
    nc = _cache["nc"]

    in_maps = _prep_in_maps(inputs)
    res = bass_utils.run_bass_kernel_spmd(
        nc, in_maps, core_ids=list(range(NCORES)))
    kernel._last_results = res

    meanT = np.empty((D, B), np.float32)
    stdT = np.empty((D, B), np.float32)
    for c in range(NCORES):
        rm = res.results[c]["mean"]
        rs = res.results[c]["std"]
        for j in range(JCH):
            s = 8 * j + c
            meanT[128 * s:128 * (s + 1)] = rm[128 * j:128 * (j + 1)]
            stdT[128 * s:128 * (s + 1)] = rs[128 * j:128 * (j + 1)]
    mean = np.ascontiguousarray(meanT.T).reshape(B, 64, 64)
    std = np.ascontiguousarray(stdT.T).reshape(B, 64, 64)
    return mean, std


# revision 16
# speedup vs baseline: 1.0676x; 1.0676x over previous
"""Bayesian masked 2-layer MLP (MC mean/std) on 8 Trainium2 NeuronCores.

Strategy (tensor-parallel over out-features, per sharding hint):
- The MADE mask is block-upper-triangular at 64 granularity; at 128x128
  "superblock" granularity, out-superblock s only contracts input features
  >= 128*s (plus one zeroed 64x64 corner on the diagonal tile). We skip the
  masked region structurally: weights/eps are shipped pre-masked and only
  the live K-range is read/computed.
- 32 out-feature superblocks; core c owns s in {c, 8+c, 16+c, 24+c}
  (chunk j -> s = 8j + c). To keep one SPMD program for all cores, chunk j
  contracts K-tiles t in [8j, 32) on every core (small zero-padding for
  cores whose s > 8j).
- Layer 1 uses the two-matmul split  x @ W1_k^T = x @ mu1^T + x @ (sig1*eps1_k)^T
  so the mu1 matmul runs once (U), not per MC sample, and no W1 assembly
  add is needed. Layer 2 assembles W2_k = mu2 + sig2*eps2_k on the vector
  engine (mu2/sig resident in SBUF).
- h_k is produced feature-major ([feat, batch]) directly by making weights
  the stationary matmul operand, so layer 2 consumes it with no transpose.
  Shards are AllGathered across the 8 cores per MC sample.
- mean/std (ddof=1) over the 5 samples computed on-device (two-pass).

Epsilons: the reference draws them with jax.random inside the model; they
are reproduced here with the identical jax calls (functionally pure ->
identical values on this jax stack) and streamed in as inputs.
"""
import os
import sys

for _p in ("/opt/trn_rl_repo", "/root/.axon_site/_ro/trn_rl_repo"):
    if _p not in sys.path and os.path.isdir(_p):
        sys.path.append(_p)

import numpy as np

B = 64          # batch
D = 4096        # flattened feature dim
NMC = 5         # MC samples
NCORES = 8
NSB = 32        # superblocks of 128 out-features
JCH = 4         # superblock chunks per core
T0 = [0, 8, 16, 24]          # first K-tile per chunk (uniform across cores)
NT = [32, 24, 16, 8]         # K-tiles per chunk
TILE_OFF = [0, 32, 56, 72]   # tile offset of chunk j in the packed [80,...]
NTILES = 80
GRP = 32                     # K-tiles per DMA/DVE group
BF16 = True                  # bf16 matmul operands (PSUM accum stays fp32)

_cache = {}


# ---------------------------------------------------------------- device code

def _build_bass():
    from concourse import bacc, tile
    import concourse.mybir as mybir

    f32 = mybir.dt.float32
    bf16 = mybir.dt.bfloat16
    wdt = bf16 if BF16 else f32
    nc = bacc.Bacc("TRN2", target_bir_lowering=False, debug=False,
                   num_devices=NCORES)

    def din(name, shape, dt=f32):
        return nc.dram_tensor(name, shape, dt, kind="ExternalInput").ap()

    xT = din("xT", [128, 32 * B], wdt)
    w1mu = din("w1mu", [128, NTILES * 128], wdt)
    w2mu = din("w2mu", [128, NTILES * 128], wdt)
    e1 = din("e1", [NMC, 128, NTILES * 128], wdt)
    e2 = din("e2", [NMC, 128, NTILES * 128], wdt)
    bmu1 = din("bmu1", [128, JCH])
    brho1 = din("brho1", [128, JCH])
    bmu2 = din("bmu2", [128, JCH])
    brho2 = din("brho2", [128, JCH])
    eb1 = din("eb1", [NMC, 128, JCH])
    eb2 = din("eb2", [NMC, 128, JCH])
    mean_o = nc.dram_tensor("mean", [JCH * 128, B], f32, kind="ExternalOutput").ap()
    std_o = nc.dram_tensor("std", [JCH * 128, B], f32, kind="ExternalOutput").ap()

    EXP = mybir.ActivationFunctionType.Exp
    LN = mybir.ActivationFunctionType.Ln
    RELU = mybir.ActivationFunctionType.Relu
    SQRT = mybir.ActivationFunctionType.Sqrt
    COPY = mybir.ActivationFunctionType.Copy
    MULT = mybir.AluOpType.mult

    # groups of K-tiles: (chunk j, packed tile offset, group size, t of first)
    groups = []
    for j in range(JCH):
        t = T0[j]
        while t < 32:
            g = min(GRP, 32 - t)
            groups.append((j, TILE_OFF[j] + (t - T0[j]), g, t))
            t += g

    with tile.TileContext(nc) as tc:
        with (
            tc.tile_pool(name="const", bufs=1) as cpool,
            tc.tile_pool(name="stream", bufs=3) as spool,
            tc.tile_pool(name="e1stream", bufs=5) as e1pool,
            tc.tile_pool(name="e2stream", bufs=3) as e2pool,
            tc.tile_pool(name="assemble", bufs=3) as apool,
            tc.tile_pool(name="small", bufs=3) as mpool,
            tc.tile_pool(name="hts", bufs=3) as hpool,
            tc.tile_pool(name="htf", bufs=3) as fpool,
            tc.tile_pool(name="psum", bufs=8, space="PSUM") as ppool,
            tc.tile_pool(name="dram", bufs=NMC, space="DRAM") as dpool,
        ):
            # ---- resident tiles
            xts = cpool.tile([128, 32 * B], wdt)
            mu2s = cpool.tile([128, NTILES * 128], wdt)
            U = cpool.tile([128, JCH * B], f32)            # 1KB/part
            y = cpool.tile([128, NMC * JCH * B], f32)      # 5KB/part
            bc = cpool.tile([128, 8 * JCH], f32)  # bmu1|bmu2|sigb1|sigb2 ...
            ebc1 = cpool.tile([128, NMC * JCH], f32)
            ebc2 = cpool.tile([128, NMC * JCH], f32)

            nc.sync.dma_start(xts[:], xT[:, :])
            nc.sync.dma_start(ebc1[:].rearrange("p (k j) -> p k j", k=NMC),
                              eb1.rearrange("k p j -> p k j"))
            nc.sync.dma_start(ebc2[:].rearrange("p (k j) -> p k j", k=NMC),
                              eb2.rearrange("k p j -> p k j"))
            nc.sync.dma_start(bc[:, 0:JCH], bmu1[:, :])
            nc.sync.dma_start(bc[:, JCH:2 * JCH], bmu2[:, :])
            # softplus(brho) -> bc[:, 2J:3J], bc[:, 3J:4J]
            tmpb = mpool.tile([128, 2 * JCH], f32, tag="tmpb")
            nc.sync.dma_start(tmpb[:, 0:JCH], brho1[:, :])
            nc.sync.dma_start(tmpb[:, JCH:2 * JCH], brho2[:, :])
            # softplus(x) = Ln(Exp(x) + 1)  (Softplus has no ACT table on gen3)
            tmpe = mpool.tile([128, 2 * JCH], f32, tag="tmpe")
            nc.scalar.activation(tmpe[:], tmpb[:], EXP)
            nc.scalar.activation(bc[:, 2 * JCH:3 * JCH], tmpe[:, 0:JCH], LN,
                                 bias=1.0)
            nc.scalar.activation(bc[:, 3 * JCH:4 * JCH], tmpe[:, JCH:2 * JCH],
                                 LN, bias=1.0)

            # ---- sigma = softplus(rho) resident; mu2 resident
            nc.gpsimd.dma_start(mu2s[:], w2mu[:, :])

            # ---- U = x @ mu1^T  (feature-major psum [outfeat, batch])
            pU = ppool.tile([128, JCH * B], f32, tag="ps")
            for (j, off, g, t) in groups:
                mt = spool.tile([128, GRP * 128], wdt, tag="ld")
                nc.gpsimd.dma_start(
                    mt[:, : g * 128],
                    w1mu[:, off * 128:(off + g) * 128],
                )
                for i in range(g):
                    tt = t + i
                    nc.tensor.matmul(
                        pU[:, j * B:(j + 1) * B],
                        mt[:, i * 128:(i + 1) * 128],
                        xts[:, tt * B:(tt + 1) * B],
                        start=(tt == T0[j]),
                        stop=(tt == 31),
                    )
            nc.scalar.activation(U[:], pU[:], COPY)

            # ---- per-sample phases
            def bias_k(k, which):
                # b_k = bmu + sigb * eb_k   -> [128, JCH] tile
                bt = mpool.tile([128, JCH], f32, tag="bk")
                ebc = ebc1 if which == 1 else ebc2
                mu_off = 0 if which == 1 else JCH
                sg_off = 2 * JCH if which == 1 else 3 * JCH
                nc.vector.tensor_tensor(
                    bt[:], ebc[:, k * JCH:(k + 1) * JCH],
                    bc[:, sg_off:sg_off + JCH], MULT)
                nc.vector.tensor_add(bt[:], bt[:], bc[:, mu_off:mu_off + JCH])
                return bt

            def layer1(k):
                b1k = bias_k(k, 1)
                hts = hpool.tile([128, JCH * B], wdt, tag="hts")
                pV = ppool.tile([128, JCH * B], f32, tag="ps")
                for (j, off, g, t) in groups:
                    et = e1pool.tile([128, GRP * 128], wdt, tag="e1")
                    nc.sync.dma_start(
                        et[:, : g * 128],
                        e1[k][:, off * 128:(off + g) * 128],
                    )
                    for i in range(g):
                        tt = t + i
                        nc.tensor.matmul(
                            pV[:, j * B:(j + 1) * B],
                            et[:, i * 128:(i + 1) * 128],
                            xts[:, tt * B:(tt + 1) * B],
                            start=(tt == T0[j]),
                            stop=(tt == 31),
                        )
                pre = mpool.tile([128, JCH * B], f32, tag="pre")
                nc.vector.tensor_add(pre[:], pV[:], U[:])
                for j in range(JCH):
                    nc.scalar.activation(hts[:, j * B:(j + 1) * B],
                                         pre[:, j * B:(j + 1) * B],
                                         RELU, bias=b1k[:, j:j + 1])
                return hts

            def gather(k, hts):
                gin = dpool.tile([JCH * 128, B], wdt, tag="gin")
                gout = dpool.tile([NSB * 128, B], wdt, tag="gout")
                nc.sync.dma_start(
                    gin.rearrange("(j p) n -> p j n", p=128),
                    hts[:].rearrange("p (j n) -> p j n", j=JCH),
                )
                nc.gpsimd.collective_compute(
                    "AllGather",
                    mybir.AluOpType.bypass,
                    ins=[gin.opt()],
                    outs=[gout.opt()],
                    replica_groups=[list(range(NCORES))],
                )
                htf = fpool.tile([128, NSB * B], wdt, tag="htf")
                # gout rows = 512*r + 128*j + p  <->  global K-tile t = 8j + r
                # htf free layout: chunk t at cols [t*B, (t+1)*B) = (j r n);
                # DMA APs are limited to 3 dims, so one DMA per source rank.
                for r in range(NCORES):
                    nc.sync.dma_start(
                        htf[:].rearrange("p (j rn) -> p j rn", j=JCH)[
                            :, :, r * B:(r + 1) * B],
                        gout[512 * r:512 * (r + 1)].rearrange(
                            "(j p) n -> p j n", p=128),
                    )
                return htf

            def layer2(k, htf):
                b2k = bias_k(k, 2)
                pY = ppool.tile([128, JCH * B], f32, tag="ps")
                for (j, off, g, t) in groups:
                    et = e2pool.tile([128, GRP * 128], wdt, tag="e2")
                    nc.gpsimd.dma_start(
                        et[:, : g * 128],
                        e2[k][:, off * 128:(off + g) * 128],
                    )
                    wt = apool.tile([128, GRP * 128], wdt, tag="RW")
                    nc.vector.tensor_add(
                        wt[:, : g * 128], et[:, : g * 128],
                        mu2s[:, off * 128:(off + g) * 128])
                    for i in range(g):
                        tt = t + i
                        nc.tensor.matmul(
                            pY[:, j * B:(j + 1) * B],
                            wt[:, i * 128:(i + 1) * 128],
                            htf[:, tt * B:(tt + 1) * B],
                            start=(tt == T0[j]),
                            stop=(tt == 31),
                        )
                for j in range(JCH):
                    nc.vector.tensor_scalar_add(
                        y[:, (k * JCH + j) * B:(k * JCH + j + 1) * B],
                        pY[:, j * B:(j + 1) * B], b2k[:, j:j + 1])

            # software pipeline: L1(k+1) overlaps gather/L2 of sample k
            hts = layer1(0)
            htfs = [None] * NMC
            for k in range(NMC):
                htfs[k] = gather(k, hts)
                if k + 1 < NMC:
                    hts = layer1(k + 1)
                layer2(k, htfs[k])

            # ---- mean / std over samples
            def ysl(k, j):
                return y[:, (k * JCH + j) * B:(k * JCH + j + 1) * B]

            for j in range(JCH):
                m = mpool.tile([128, B], f32, tag="m")
                t1 = mpool.tile([128, B], f32, tag="t1")
                t2 = mpool.tile([128, B], f32, tag="t2")
                nc.vector.tensor_add(t1[:], ysl(0, j), ysl(1, j))
                nc.vector.tensor_add(t2[:], ysl(2, j), ysl(3, j))
                nc.vector.tensor_add(t1[:], t1[:], t2[:])
                nc.vector.tensor_add(t1[:], t1[:], ysl(4, j))
                nc.vector.tensor_scalar_mul(m[:], t1[:], 1.0 / NMC)
                nc.sync.dma_start(mean_o.rearrange("(j p) n -> p j n", p=128)[:, j],
                                  m[:])
                acc = mpool.tile([128, B], f32, tag="acc")
                d = mpool.tile([128, B], f32, tag="d")
                nc.vector.tensor_sub(d[:], ysl(0, j), m[:])
                nc.vector.tensor_tensor(acc[:], d[:], d[:], MULT)
                for k in range(1, NMC):
                    dk = mpool.tile([128, B], f32, tag="d")
                    sq = mpool.tile([128, B], f32, tag="sq")
                    nc.vector.tensor_sub(dk[:], ysl(k, j), m[:])
                    nc.vector.tensor_tensor(sq[:], dk[:], dk[:], MULT)
                    nc.vector.tensor_add(acc[:], acc[:], sq[:])
                nc.vector.tensor_scalar_mul(acc[:], acc[:], 1.0 / (NMC - 1))
                std_t = mpool.tile([128, B], f32, tag="stdt")
                nc.scalar.activation(std_t[:], acc[:], SQRT)
                nc.sync.dma_start(std_o.rearrange("(j p) n -> p j n", p=128)[:, j],
                                  std_t[:])

    nc.compile()
    return nc


# ---------------------------------------------------------------- host prep

def _gen_eps():
    """Reproduce the reference's jax.random draws (identical calls/keys)."""
    import jax

    keys = jax.random.split(jax.random.key(42), NMC * 4).reshape(NMC, 4)

    def one_mc_eps(k):
        return (
            jax.random.normal(k[0], (D, D)),
            jax.random.normal(k[1], (D,)),
            jax.random.normal(k[2], (D, D)),
            jax.random.normal(k[3], (D,)),
        )

    ew1, ebs1, ew2, ebs2 = jax.vmap(one_mc_eps)(keys)
    return (np.asarray(ew1), np.asarray(ebs1),
            np.asarray(ew2), np.asarray(ebs2))


def _pack_weight(w, c, masked):
    """Pack [D, D] (out, in) -> SBUF image [128, NTILES*128]: partition p =
    within-K-tile row, free = (tile, outfeat-local). Chunk j covers
    superblock s=8j+c, K-tiles t in [T0[j], 32); tiles t < s stay zero
    (mask padding); diagonal corner zeroed if masked."""
    out = np.zeros((NTILES, 128, 128), np.float32)
    blk = w.reshape(NSB, 128, NSB, 128)  # [s_out, f, t_in, p]
    for j in range(JCH):
        s = 8 * j + c
        # copy tiles t in [s, 32): blk[s, :, s:32, :] -> [t, p, f]
        src = blk[s, :, s:32, :].transpose(1, 2, 0)
        out[TILE_OFF[j] + (s - T0[j]): TILE_OFF[j] + NT[j]] = src
        if masked:
            out[TILE_OFF[j] + (s - T0[j]), 0:64, 64:128] = 0.0
    return np.ascontiguousarray(out.transpose(1, 0, 2)).reshape(128, NTILES * 128)


def _pack_bias(v, c):
    """[D] -> [128, JCH]: column j = superblock 8j+c."""
    return np.ascontiguousarray(
        v.reshape(NSB, 128)[[8 * j + c for j in range(JCH)], :].T
    ).astype(np.float32)


def _prep_in_maps(inputs):
    import ml_dtypes
    wdt = ml_dtypes.bfloat16 if BF16 else np.float32

    x = np.asarray(inputs["x"], np.float32)
    xTf = x.reshape(B, D).T.reshape(32, 128, B)
    xT = np.ascontiguousarray(xTf.transpose(1, 0, 2)).reshape(128, 32 * B).astype(wdt)

    if "eps" not in _cache:
        _cache["eps"] = _gen_eps()
    ew1, ebs1, ew2, ebs2 = _cache["eps"]

    mu1 = np.asarray(inputs["weight_mu1"], np.float32)
    rho1 = np.asarray(inputs["weight_rho1"], np.float32)
    mu2 = np.asarray(inputs["weight_mu2"], np.float32)
    rho2 = np.asarray(inputs["weight_rho2"], np.float32)
    sig1 = np.log1p(np.exp(rho1))
    sig2 = np.log1p(np.exp(rho2))

    in_maps = []
    for c in range(NCORES):
        m = {
            "xT": xT,
            "w1mu": _pack_weight(mu1, c, masked=True).astype(wdt),
            "w2mu": _pack_weight(mu2, c, masked=True).astype(wdt),
            "e1": np.stack([_pack_weight(ew1[k] * sig1, c, masked=True)
                            .astype(wdt) for k in range(NMC)]),
            "e2": np.stack([_pack_weight(ew2[k] * sig2, c, masked=True)
                            .astype(wdt) for k in range(NMC)]),
            "bmu1": _pack_bias(np.asarray(inputs["bias_mu1"], np.float32), c),
            "brho1": _pack_bias(np.asarray(inputs["bias_rho1"], np.float32), c),
            "bmu2": _pack_bias(np.asarray(inputs["bias_mu2"], np.float32), c),
            "brho2": _pack_bias(np.asarray(inputs["bias_rho2"], np.float32), c),
            "eb1": np.stack([_pack_bias(ebs1[k], c) for k in range(NMC)]),
            "eb2": np.stack([_pack_bias(ebs2[k], c) for k in range(NMC)]),
        }
        in_maps.append(m)
    return in_maps


def kernel(**inputs):
    import concourse.bass_utils as bass_utils

    if "nc" not in _cache:
        _cache["nc"] = _build_bass()
    nc = _cache["nc"]

    in_maps = _prep_in_maps(inputs)
    res = bass_utils.run_bass_kernel_spmd(
        nc, in_maps, core_ids=list(range(NCORES)))
    kernel._last_results = res

    meanT = np.empty((D, B), np.float32)
    stdT = np.empty((D, B), np.float32)
    for c in range(NCORES):
        rm = res.results[c]["mean"]
        rs = res.results[c]["std"]
        for j in range(JCH):
            s = 8 * j + c
            meanT[128 * s:128 * (s + 1)] = rm[128 * j:128 * (j + 1)]
            stdT[128 * s:128 * (s + 1)] = rs[128 * j:128 * (j + 1)]
    mean = np.ascontiguousarray(meanT.T).reshape(B, 64, 64)
    std = np.ascontiguousarray(stdT.T).reshape(B, 64, 64)
    return mean, std


# revision 19
# speedup vs baseline: 1.1715x; 1.0973x over previous
"""Bayesian masked 2-layer MLP (MC mean/std) on 8 Trainium2 NeuronCores.

Strategy (tensor-parallel over out-features, per sharding hint):
- The MADE mask is block-upper-triangular at 64 granularity; at 128x128
  "superblock" granularity, out-superblock s only contracts input features
  >= 128*s (plus one zeroed 64x64 corner on the diagonal tile). We skip the
  masked region structurally: weights/eps are shipped pre-masked and only
  the live K-range is read/computed.
- 32 out-feature superblocks; core c owns s in {c, 8+c, 16+c, 24+c}
  (chunk j -> s = 8j + c). To keep one SPMD program for all cores, chunk j
  contracts K-tiles t in [8j, 32) on every core (small zero-padding for
  cores whose s > 8j).
- Layer 1 uses the two-matmul split  x @ W1_k^T = x @ mu1^T + x @ (sig1*eps1_k)^T
  so the mu1 matmul runs once (U), not per MC sample, and no W1 assembly
  add is needed. Layer 2 assembles W2_k = mu2 + sig2*eps2_k on the vector
  engine (mu2/sig resident in SBUF).
- h_k is produced feature-major ([feat, batch]) directly by making weights
  the stationary matmul operand, so layer 2 consumes it with no transpose.
  Shards are AllGathered across the 8 cores per MC sample.
- mean/std (ddof=1) over the 5 samples computed on-device (two-pass).

Epsilons: the reference draws them with jax.random inside the model; they
are reproduced here with the identical jax calls (functionally pure ->
identical values on this jax stack) and streamed in as inputs.
"""
import os
import sys

for _p in ("/opt/trn_rl_repo", "/root/.axon_site/_ro/trn_rl_repo"):
    if _p not in sys.path and os.path.isdir(_p):
        sys.path.append(_p)

import numpy as np

B = 64          # batch
D = 4096        # flattened feature dim
NMC = 5         # MC samples
NCORES = 8
NSB = 32        # superblocks of 128 out-features
JCH = 4         # superblock chunks per core
T0 = [0, 8, 16, 24]          # first K-tile per chunk (uniform across cores)
NT = [32, 24, 16, 8]         # K-tiles per chunk
TILE_OFF = [0, 32, 56, 72]   # tile offset of chunk j in the packed [80,...]
NTILES = 80
OFF2T = []                   # layer-2 t-major packed tile offset per K-tile t
_acc = 0
for _t in range(32):
    OFF2T.append(_acc)
    _acc += _t // 8 + 1
GRP = 32                     # K-tiles per DMA/DVE group
BF16 = True                  # bf16 matmul operands (PSUM accum stays fp32)

_cache = {}


# ---------------------------------------------------------------- device code

def _build_bass():
    from concourse import bacc, tile
    import concourse.mybir as mybir

    f32 = mybir.dt.float32
    bf16 = mybir.dt.bfloat16
    wdt = bf16 if BF16 else f32
    nc = bacc.Bacc("TRN2", target_bir_lowering=False, debug=False,
                   num_devices=NCORES)

    def din(name, shape, dt=f32):
        return nc.dram_tensor(name, shape, dt, kind="ExternalInput").ap()

    xT = din("xT", [128, 32 * B], wdt)
    w1mu = din("w1mu", [128, NTILES * 128], wdt)
    w2mu = din("w2mu", [128, NTILES * 128], wdt)
    e1 = din("e1", [NMC, 128, NTILES * 128], wdt)
    e2 = din("e2", [NMC, 128, NTILES * 128], wdt)
    bmu1 = din("bmu1", [128, JCH])
    brho1 = din("brho1", [128, JCH])
    eb1 = din("eb1", [NMC, 128, JCH])
    bmu2f = din("bmu2f", [1, JCH * 128])
    brho2f = din("brho2f", [1, JCH * 128])
    eb2f = din("eb2f", [NMC, 1, JCH * 128])
    mean_o = nc.dram_tensor("mean", [B, JCH * 128], f32, kind="ExternalOutput").ap()
    std_o = nc.dram_tensor("std", [B, JCH * 128], f32, kind="ExternalOutput").ap()

    EXP = mybir.ActivationFunctionType.Exp
    LN = mybir.ActivationFunctionType.Ln
    RELU = mybir.ActivationFunctionType.Relu
    SQRT = mybir.ActivationFunctionType.Sqrt
    COPY = mybir.ActivationFunctionType.Copy
    MULT = mybir.AluOpType.mult

    # groups of K-tiles: (chunk j, packed tile offset, group size, t of first)
    groups = []
    for j in range(JCH):
        t = T0[j]
        while t < 32:
            g = min(GRP, 32 - t)
            groups.append((j, TILE_OFF[j] + (t - T0[j]), g, t))
            t += g

    with tile.TileContext(nc) as tc:
        with (
            tc.tile_pool(name="const", bufs=1) as cpool,
            tc.tile_pool(name="stream", bufs=3) as spool,
            tc.tile_pool(name="e1stream", bufs=5) as e1pool,
            tc.tile_pool(name="e2stream", bufs=3) as e2pool,
            tc.tile_pool(name="assemble", bufs=3) as apool,
            tc.tile_pool(name="small", bufs=3) as mpool,
            tc.tile_pool(name="hts", bufs=3) as hpool,
            tc.tile_pool(name="htf", bufs=3) as fpool,
            tc.tile_pool(name="psum", bufs=4, space="PSUM") as ppool,
            tc.tile_pool(name="dram", bufs=NMC, space="DRAM") as dpool,
        ):
            # ---- resident tiles
            xts = cpool.tile([128, 32 * B], wdt)
            mu2s = cpool.tile([128, NTILES * 128], wdt)
            U = cpool.tile([128, JCH * B], f32)
            y = cpool.tile([64, NMC * JCH * 128], f32)     # batch-major L2 out
            bc = cpool.tile([128, 4 * JCH], f32)           # bmu1 | sigb1
            ebc1 = cpool.tile([128, NMC * JCH], f32)
            onesb = cpool.tile([1, B], wdt)
            bc2 = cpool.tile([1, 3 * JCH * 128], f32)      # bmu2f|sigb2f|scratch
            ebc2 = cpool.tile([1, NMC * JCH * 128], f32)
            b2kb = cpool.tile([1, NMC * JCH * 128], wdt)   # per-k bf16 bias rows
            nc.gpsimd.memset(onesb[:], 1.0)

            nc.sync.dma_start(xts[:], xT[:, :])
            nc.sync.dma_start(ebc1[:].rearrange("p (k j) -> p k j", k=NMC),
                              eb1.rearrange("k p j -> p k j"))
            nc.sync.dma_start(ebc2[:].rearrange("o (k f) -> o k f", k=NMC),
                              eb2f.rearrange("k o f -> o k f"))
            nc.sync.dma_start(bc[:, 0:JCH], bmu1[:, :])
            nc.sync.dma_start(bc2[:, 0:JCH * 128], bmu2f[:, :])
            tmpb = mpool.tile([128, JCH], f32, tag="tmpb")
            nc.sync.dma_start(tmpb[:], brho1[:, :])
            # softplus(x) = Ln(Exp(x) + 1)  (Softplus has no ACT table on gen3)
            tmpe = mpool.tile([128, JCH], f32, tag="tmpe")
            nc.scalar.activation(tmpe[:], tmpb[:], EXP)
            nc.scalar.activation(bc[:, 2 * JCH:3 * JCH], tmpe[:], LN, bias=1.0)
            # b2 free-layout: sigb2f then all 5 bf16 bias rows up front
            nc.sync.dma_start(bc2[:, 2 * JCH * 128:3 * JCH * 128],
                              brho2f[:, :])
            nc.scalar.activation(bc2[:, JCH * 128:2 * JCH * 128],
                                 bc2[:, 2 * JCH * 128:3 * JCH * 128], EXP)
            nc.scalar.activation(bc2[:, JCH * 128:2 * JCH * 128],
                                 bc2[:, JCH * 128:2 * JCH * 128], LN, bias=1.0)
            for k in range(NMC):
                nc.vector.tensor_tensor(
                    bc2[:, 2 * JCH * 128:3 * JCH * 128],
                    ebc2[:, k * JCH * 128:(k + 1) * JCH * 128],
                    bc2[:, JCH * 128:2 * JCH * 128], MULT)
                nc.vector.tensor_add(
                    b2kb[:, k * JCH * 128:(k + 1) * JCH * 128],
                    bc2[:, 2 * JCH * 128:3 * JCH * 128], bc2[:, 0:JCH * 128])

            # ---- sigma = softplus(rho) resident; mu2 resident
            nc.gpsimd.dma_start(mu2s[:], w2mu[:, :])

            # ---- U = x @ mu1^T  (feature-major psum [outfeat, batch])
            pU = ppool.tile([128, JCH * B], f32, tag="ps")
            for (j, off, g, t) in groups:
                mt = spool.tile([128, GRP * 128], wdt, tag="ld")
                nc.gpsimd.dma_start(
                    mt[:, : g * 128],
                    w1mu[:, off * 128:(off + g) * 128],
                )
                for i in range(g):
                    tt = t + i
                    nc.tensor.matmul(
                        pU[:, j * B:(j + 1) * B],
                        mt[:, i * 128:(i + 1) * 128],
                        xts[:, tt * B:(tt + 1) * B],
                        start=(tt == T0[j]),
                        stop=(tt == 31),
                    )
            nc.scalar.activation(U[:], pU[:], COPY)

            # ---- per-sample phases
            def bias_k(k):
                # b1_k = bmu1 + sigb1 * eb1_k   -> [128, JCH] tile
                bt = mpool.tile([128, JCH], f32, tag="bk")
                nc.vector.tensor_tensor(
                    bt[:], ebc1[:, k * JCH:(k + 1) * JCH],
                    bc[:, 2 * JCH:3 * JCH], MULT)
                nc.vector.tensor_add(bt[:], bt[:], bc[:, 0:JCH])
                return bt

            def layer1(k):
                b1k = bias_k(k)
                hts = hpool.tile([128, JCH * B], wdt, tag="hts")
                pV = ppool.tile([128, JCH * B], f32, tag="ps")
                for (j, off, g, t) in groups:
                    et = e1pool.tile([128, GRP * 128], wdt, tag="e1")
                    nc.sync.dma_start(
                        et[:, : g * 128],
                        e1[k][:, off * 128:(off + g) * 128],
                    )
                    for i in range(g):
                        tt = t + i
                        nc.tensor.matmul(
                            pV[:, j * B:(j + 1) * B],
                            et[:, i * 128:(i + 1) * 128],
                            xts[:, tt * B:(tt + 1) * B],
                            start=(tt == T0[j]),
                            stop=(tt == 31),
                        )
                pre = mpool.tile([128, JCH * B], f32, tag="pre")
                nc.vector.tensor_add(pre[:], pV[:], U[:])
                for j in range(JCH):
                    nc.scalar.activation(hts[:, j * B:(j + 1) * B],
                                         pre[:, j * B:(j + 1) * B],
                                         RELU, bias=b1k[:, j:j + 1])
                return hts

            def gather(k, hts):
                gin = dpool.tile([JCH * 128, B], wdt, tag="gin")
                gout = dpool.tile([NSB * 128, B], wdt, tag="gout")
                nc.sync.dma_start(
                    gin.rearrange("(j p) n -> p j n", p=128),
                    hts[:].rearrange("p (j n) -> p j n", j=JCH),
                )
                nc.gpsimd.collective_compute(
                    "AllGather",
                    mybir.AluOpType.bypass,
                    ins=[gin.opt()],
                    outs=[gout.opt()],
                    replica_groups=[list(range(NCORES))],
                )
                htf = fpool.tile([128, NSB * B], wdt, tag="htf")
                # gout rows = 512*r + 128*j + p  <->  global K-tile t = 8j + r
                # htf free layout: chunk t at cols [t*B, (t+1)*B) = (j r n);
                # DMA APs are limited to 3 dims, so one DMA per source rank.
                for r in range(NCORES):
                    nc.sync.dma_start(
                        htf[:].rearrange("p (j rn) -> p j rn", j=JCH)[
                            :, :, r * B:(r + 1) * B],
                        gout[512 * r:512 * (r + 1)].rearrange(
                            "(j p) n -> p j n", p=128),
                    )
                return htf

            def layer2(k, htf):
                # batch-stationary: lhsT = htf_t (one 64-col LDW per K-tile),
                # rhs = t-major prefix-packed W2 tiles (N = 128*(t//8+1)).
                # K=1 rank-1 bias matmul initializes the psum bank (start).
                pY = ppool.tile([64, JCH * 128], f32, tag="ps2")
                nc.tensor.matmul(
                    pY[:], onesb[:],
                    b2kb[:, k * JCH * 128:(k + 1) * JCH * 128],
                    start=True, stop=False)
                for a in range(4):  # octile of K-tiles: t in [8a, 8a+8)
                    na = a + 1
                    off = OFF2T[8 * a]
                    cols = 8 * na * 128
                    et = e2pool.tile([128, GRP * 128], wdt, tag="e2")
                    nc.gpsimd.dma_start(
                        et[:, :cols],
                        e2[k][:, off * 128: off * 128 + cols],
                    )
                    wt = apool.tile([128, GRP * 128], wdt, tag="RW")
                    nc.vector.tensor_add(
                        wt[:, :cols], et[:, :cols],
                        mu2s[:, off * 128: off * 128 + cols])
                    for i in range(8):
                        t = 8 * a + i
                        nc.tensor.matmul(
                            pY[:, 0: na * 128],
                            htf[:, t * B:(t + 1) * B],
                            wt[:, i * na * 128:(i + 1) * na * 128],
                            start=False,
                            stop=(t == 31),
                        )
                nc.vector.tensor_copy(
                    y[:, k * JCH * 128:(k + 1) * JCH * 128], pY[:])

            # software pipeline: L1(k+1) overlaps gather/L2 of sample k
            hts = layer1(0)
            htfs = [None] * NMC
            for k in range(NMC):
                htfs[k] = gather(k, hts)
                if k + 1 < NMC:
                    hts = layer1(k + 1)
                layer2(k, htfs[k])

            # ---- mean / std over samples
            def ysl(k, j):
                return y[:, (k * JCH + j) * 128:(k * JCH + j + 1) * 128]

            for j in range(JCH):
                m = mpool.tile([64, 128], f32, tag="m")
                t1 = mpool.tile([64, 128], f32, tag="t1")
                t2 = mpool.tile([64, 128], f32, tag="t2")
                nc.vector.tensor_add(t1[:], ysl(0, j), ysl(1, j))
                nc.vector.tensor_add(t2[:], ysl(2, j), ysl(3, j))
                nc.vector.tensor_add(t1[:], t1[:], t2[:])
                nc.vector.tensor_add(t1[:], t1[:], ysl(4, j))
                nc.vector.tensor_scalar_mul(m[:], t1[:], 1.0 / NMC)
                nc.sync.dma_start(mean_o[:, j * 128:(j + 1) * 128], m[:])
                acc = mpool.tile([64, 128], f32, tag="acc")
                d = mpool.tile([64, 128], f32, tag="d")
                nc.vector.tensor_sub(d[:], ysl(0, j), m[:])
                nc.vector.tensor_tensor(acc[:], d[:], d[:], MULT)
                for k in range(1, NMC):
                    dk = mpool.tile([64, 128], f32, tag="d")
                    sq = mpool.tile([64, 128], f32, tag="sq")
                    nc.vector.tensor_sub(dk[:], ysl(k, j), m[:])
                    nc.vector.tensor_tensor(sq[:], dk[:], dk[:], MULT)
                    nc.vector.tensor_add(acc[:], acc[:], sq[:])
                nc.vector.tensor_scalar_mul(acc[:], acc[:], 1.0 / (NMC - 1))
                std_t = mpool.tile([64, 128], f32, tag="stdt")
                nc.scalar.activation(std_t[:], acc[:], SQRT)
                nc.sync.dma_start(std_o[:, j * 128:(j + 1) * 128], std_t[:])

    nc.compile()
    return nc


# ---------------------------------------------------------------- host prep

def _gen_eps():
    """Reproduce the reference's jax.random draws (identical calls/keys)."""
    import jax

    keys = jax.random.split(jax.random.key(42), NMC * 4).reshape(NMC, 4)

    def one_mc_eps(k):
        return (
            jax.random.normal(k[0], (D, D)),
            jax.random.normal(k[1], (D,)),
            jax.random.normal(k[2], (D, D)),
            jax.random.normal(k[3], (D,)),
        )

    ew1, ebs1, ew2, ebs2 = jax.vmap(one_mc_eps)(keys)
    return (np.asarray(ew1), np.asarray(ebs1),
            np.asarray(ew2), np.asarray(ebs2))


def _pack_weight(w, c, masked):
    """Pack [D, D] (out, in) -> SBUF image [128, NTILES*128]: partition p =
    within-K-tile row, free = (tile, outfeat-local). Chunk j covers
    superblock s=8j+c, K-tiles t in [T0[j], 32); tiles t < s stay zero
    (mask padding); diagonal corner zeroed if masked."""
    out = np.zeros((NTILES, 128, 128), np.float32)
    blk = w.reshape(NSB, 128, NSB, 128)  # [s_out, f, t_in, p]
    for j in range(JCH):
        s = 8 * j + c
        # copy tiles t in [s, 32): blk[s, :, s:32, :] -> [t, p, f]
        src = blk[s, :, s:32, :].transpose(1, 2, 0)
        out[TILE_OFF[j] + (s - T0[j]): TILE_OFF[j] + NT[j]] = src
        if masked:
            out[TILE_OFF[j] + (s - T0[j]), 0:64, 64:128] = 0.0
    return np.ascontiguousarray(out.transpose(1, 0, 2)).reshape(128, NTILES * 128)


def _pack_weight_tmajor(w, c, masked):
    """Layer-2 packing: t-major with prefix-packed active chunks. At K-tile t
    the active chunks are j in [0, t//8]; tile (j, t) holds superblock
    s=8j+c (zeros when t < s; diagonal corner zeroed if masked)."""
    out = np.zeros((NTILES, 128, 128), np.float32)
    blk = w.reshape(NSB, 128, NSB, 128)  # [s_out, f, t_in, p]
    for t in range(32):
        for j in range(t // 8 + 1):
            s = 8 * j + c
            if t >= s:
                tile = blk[s, :, t, :].T.copy()
                if masked and t == s:
                    tile[0:64, 64:128] = 0.0
                out[OFF2T[t] + j] = tile
    return np.ascontiguousarray(out.transpose(1, 0, 2)).reshape(128, NTILES * 128)


def _pack_bias(v, c):
    """[D] -> [128, JCH]: column j = superblock 8j+c."""
    return np.ascontiguousarray(
        v.reshape(NSB, 128)[[8 * j + c for j in range(JCH)], :].T
    ).astype(np.float32)


def _prep_in_maps(inputs):
    import ml_dtypes
    wdt = ml_dtypes.bfloat16 if BF16 else np.float32

    x = np.asarray(inputs["x"], np.float32)
    xTf = x.reshape(B, D).T.reshape(32, 128, B)
    xT = np.ascontiguousarray(xTf.transpose(1, 0, 2)).reshape(128, 32 * B).astype(wdt)

    if "eps" not in _cache:
        _cache["eps"] = _gen_eps()
    ew1, ebs1, ew2, ebs2 = _cache["eps"]

    mu1 = np.asarray(inputs["weight_mu1"], np.float32)
    rho1 = np.asarray(inputs["weight_rho1"], np.float32)
    mu2 = np.asarray(inputs["weight_mu2"], np.float32)
    rho2 = np.asarray(inputs["weight_rho2"], np.float32)
    sig1 = np.log1p(np.exp(rho1))
    sig2 = np.log1p(np.exp(rho2))

    in_maps = []
    for c in range(NCORES):
        m = {
            "xT": xT,
            "w1mu": _pack_weight(mu1, c, masked=True).astype(wdt),
            "w2mu": _pack_weight_tmajor(mu2, c, masked=True).astype(wdt),
            "e1": np.stack([_pack_weight(ew1[k] * sig1, c, masked=True)
                            .astype(wdt) for k in range(NMC)]),
            "e2": np.stack([_pack_weight_tmajor(ew2[k] * sig2, c, masked=True)
                            .astype(wdt) for k in range(NMC)]),
            "bmu1": _pack_bias(np.asarray(inputs["bias_mu1"], np.float32), c),
            "brho1": _pack_bias(np.asarray(inputs["bias_rho1"], np.float32), c),
            "eb1": np.stack([_pack_bias(ebs1[k], c) for k in range(NMC)]),
            "bmu2f": _pack_bias(np.asarray(inputs["bias_mu2"], np.float32),
                                c).T.reshape(1, JCH * 128).copy(),
            "brho2f": _pack_bias(np.asarray(inputs["bias_rho2"], np.float32),
                                 c).T.reshape(1, JCH * 128).copy(),
            "eb2f": np.stack([_pack_bias(ebs2[k], c).T.reshape(1, JCH * 128)
                              .copy() for k in range(NMC)]),
        }
        in_maps.append(m)
    return in_maps


def kernel(**inputs):
    import concourse.bass_utils as bass_utils

    if "nc" not in _cache:
        _cache["nc"] = _build_bass()
    nc = _cache["nc"]

    in_maps = _prep_in_maps(inputs)
    res = bass_utils.run_bass_kernel_spmd(
        nc, in_maps, core_ids=list(range(NCORES)))
    kernel._last_results = res

    mean = np.empty((B, D), np.float32)
    std = np.empty((B, D), np.float32)
    for c in range(NCORES):
        rm = res.results[c]["mean"]
        rs = res.results[c]["std"]
        for j in range(JCH):
            s = 8 * j + c
            mean[:, 128 * s:128 * (s + 1)] = rm[:, 128 * j:128 * (j + 1)]
            std[:, 128 * s:128 * (s + 1)] = rs[:, 128 * j:128 * (j + 1)]
    return mean.reshape(B, 64, 64), std.reshape(B, 64, 64)
